# revision 15
# baseline (speedup 1.0000x reference)
# Trainium2 Bass kernel for nn_ExpertLinear (MoE grouped GEMM with routing).
#
# v2 strategy (vs the replicated-weight baseline):
#   * 4-expert covering design: core c serves expert set BSET[c]; every
#     expert PAIR is inside some core's set, so each token (2 expert slots)
#     is computed and combined entirely on one core, while each core loads
#     only 4 of 8 expert weight matrices (8.4 MB fp16 vs 16.8 MB).
#   * Host-side dispatch: the routing tables are host-known, so x rows are
#     gathered/transposed into each core's stationary [P, tile, k, 128]
#     layout on the host and DMA'd as plain contiguous tiles — no Q7
#     dispatch gathers, matmuls start ~2 us into the kernel.
#   * Same-expert pairs merged: a token routed twice to expert e becomes
#     one GEMM row with gate (g0+g1)/2 and r0 == r1 in the combine.
#   * Readiness-ordered combine: tokens sorted by the tile index at which
#     their last row is evicted; combine chunks (Q7 transpose-gather from
#     the y buffer + gpsimd add + DMA out) overlap the GEMM phase, with
#     partial-range deps via y[:, :RT+1, :] slices.
import os
import numpy as np

import concourse.bacc as bacc
import concourse.bass as bass
import concourse.mybir as mybir
import concourse.tile as tile
from concourse.bass_utils import run_bass_kernel_spmd

N_TOK = 8192
TOPK = 2
N_EXP = 8
D_IN = 1024
D_OUT = 1024
NCORES = 8
TPC = N_TOK // NCORES          # tokens per core
P = 128
KT = D_IN // P                 # 8 k-tiles over d_in
NSLOT = 4                      # experts per core (covering design)
F16 = mybir.dt.float16
F32 = mybir.dt.float32
I16 = mybir.dt.int16

# Covering design: every unordered expert pair {a,b} (incl. a==b) is a
# subset of at least one block; each expert appears in exactly 4 blocks.
BLOCKS = [(0, 1, 2, 3), (4, 5, 6, 7), (0, 1, 4, 5), (2, 3, 6, 7),
          (0, 2, 4, 6), (1, 3, 5, 7), (0, 3, 4, 7), (1, 2, 5, 6)]
BSET = [frozenset(blk) for blk in BLOCKS]

CHUNKS = [256, 256, 128, 128, 128, 128]  # gather-combine chunk sizes
NGATH = sum(CHUNKS)                      # gather-combine token slots
NDIR = P                                 # merged direct-out token slots
NOUT = NGATH + NDIR                      # out slots per core (>= real tokens)

# core -> expert whose same-expert ("merged") tokens get this core's
# dedicated direct-out tile (bijection, e in BLOCKS[c])
MEXP = [0, 4, 1, 2, 6, 3, 7, 5]


def _pack16(flat):
    # [16, n/16] block (idx j at [j%16, j//16]), replicated into all eight
    # 16-partition groups — each GpSimd Q7 core reads its own copy.
    return np.ascontiguousarray(np.tile(flat.reshape(-1, 16).T, (8, 1)))


def _cores_of_pair():
    m = {}
    for x in range(N_EXP):
        for y in range(x, N_EXP):
            m[x * N_EXP + y] = [c for c in range(NCORES)
                                if x in BSET[c] and y in BSET[c]]
    return m


def _assign_counts(cnt_of):
    """Distribute gather-path tokens (by pair group) over covering cores.
    cnt_of: pair id -> token count (cross pairs + overflow merged).
    Returns n[pid][core] counts, balancing per-core loads (<= NGATH) and
    per-(core, expert) row counts toward <= 512."""
    cop = _cores_of_pair()
    load = np.zeros(NCORES, np.int64)           # tokens per core
    rcnt = np.zeros((NCORES, N_EXP), np.int64)  # rows per (core, expert)
    n = {p: np.zeros(NCORES, np.int64) for p in cnt_of}

    def rows_of(p):
        a, b = p // N_EXP, p % N_EXP
        return (a, b) if a != b else (a,)

    # most-constrained pair groups first
    groups = sorted(cnt_of, key=lambda p: (len(cop[p]), -cnt_of[p]))
    for p in groups:
        cs = cop[p]
        rexp = rows_of(p)
        for _ in range(cnt_of[p]):
            best, bc = None, None
            for c in cs:
                if load[c] >= NGATH:
                    continue
                cost = 4.0 * load[c] + sum(rcnt[c, e] for e in rexp)
                if best is None or cost < best:
                    best, bc = cost, c
            assert bc is not None, "balancer stuck"
            n[p][bc] += 1
            load[bc] += 1
            for e in rexp:
                rcnt[bc, e] += 1

    # quadratic-potential refinement: move units between covering cores
    # while it reduces  sum(rcnt^2) + WL*sum(load^2)  (keeps rcnt even ->
    # minimal shared tile counts T)
    WL = 0.3
    loads = load
    for _ in range(40):
        improved = False
        for p in groups:
            cs = cop[p]
            if len(cs) < 2:
                continue
            exps = rows_of(p)
            for cf in cs:
                for ct in cs:
                    if ct == cf:
                        continue
                    while n[p][cf] > 0 and loads[ct] < NGATH:
                        d = WL * 2.0 * (loads[ct] - loads[cf] + 1)
                        for e in exps:
                            d += 2.0 * (rcnt[ct, e] - rcnt[cf, e] + 1)
                        if d >= 0:
                            break
                        n[p][cf] -= 1
                        n[p][ct] += 1
                        loads[cf] -= 1
                        loads[ct] += 1
                        for e in exps:
                            rcnt[cf, e] -= 1
                            rcnt[ct, e] += 1
                        improved = True
        if not improved:
            break
    return n, rcnt


def _plan(te, tg):
    """Host routing plan.  te [N_TOK, 2] expert ids, tg [N_TOK, 2] gates.
    Returns (T, RTs, per_core input dicts, per-core token id tables).
    T covers slots 0..3; one extra merged direct-out tile follows them."""
    a = np.minimum(te[:, 0], te[:, 1])
    b = np.maximum(te[:, 0], te[:, 1])
    pid = (a * N_EXP + b).astype(np.int64)
    merged = a == b
    gsum = tg.sum(axis=1)

    # merged tokens of expert e -> direct-out tile on core MEXP^-1(e)
    # (up to NDIR); the overflow joins the gather path
    direct_toks = [None] * NCORES        # per core: token ids (<= NDIR)
    gather_tok = np.ones(N_TOK, bool)
    for c in range(NCORES):
        e = MEXP[c]
        toks = np.where(merged & (a == e))[0][:NDIR]
        direct_toks[c] = toks
        gather_tok[toks] = False

    cnt_of = {}
    for p in np.unique(pid[gather_tok]):
        cnt_of[int(p)] = int(((pid == p) & gather_tok).sum())
    n, rcnt = _assign_counts(cnt_of)

    # concrete token -> core assignment (tokens within a pair group are
    # interchangeable)
    core_of = np.full(N_TOK, -1, np.int64)
    for p, npc in n.items():
        toks = np.where((pid == p) & gather_tok)[0]
        base = 0
        for c in range(NCORES):
            k = int(npc[c])
            core_of[toks[base:base + k]] = c
            base += k
    assert (core_of[gather_tok] >= 0).all()

    # per-core expert -> slot (largest row count first), shared tile counts
    slots = []          # slots[c] = [expert per slot]
    cnt_cs = np.zeros((NCORES, NSLOT), np.int64)
    for c in range(NCORES):
        es = sorted(BLOCKS[c], key=lambda e: -rcnt[c, e])
        slots.append(es)
        for s, e in enumerate(es):
            cnt_cs[c, s] = rcnt[c, e]
    T = np.maximum(1, -(-cnt_cs.max(axis=0) // P))       # tiles per slot
    off_rows = np.concatenate([[0], np.cumsum(T)]) * P
    NBs = int(T.sum())                   # gather-path tiles
    NB = NBs + 1                         # + merged direct-out tile
    NP = NB * P

    per_core = []
    token_ids = []
    ready_all = np.zeros((NCORES, len(CHUNKS)), np.int64)
    for c in range(NCORES):
        slot_of = {e: s for s, e in enumerate(slots[c])}
        toks_c = np.where(core_of == c)[0]
        # rows per slot: (token, gate, partner_slot); partner -1 == closer
        rows_slot = [[] for _ in range(NSLOT)]
        for t in toks_c:
            ea, eb = int(te[t, 0]), int(te[t, 1])
            if ea == eb:
                rows_slot[slot_of[ea]].append((t, gsum[t] * 0.5, -1))
            else:
                sa, sb = slot_of[ea], slot_of[eb]
                rows_slot[sa].append((t, tg[t, 0], sb))
                rows_slot[sb].append((t, tg[t, 1], sa))
        grow_flat = np.zeros(NP, np.float32)
        src_tok = np.full(NP, -1, np.int64)
        row_of = {}                      # token -> [row indices]
        for s in range(NSLOT):
            # rows that close a token (partner slot earlier / merged) first
            rows = sorted(rows_slot[s],
                          key=lambda r: (0, r[2]) if r[2] < s else (1, 0))
            assert len(rows) <= int(T[s]) * P, (c, s, len(rows), int(T[s]))
            for i, (t, g, _ps) in enumerate(rows):
                r = int(off_rows[s]) + i
                grow_flat[r] = g
                src_tok[r] = t
                row_of.setdefault(t, []).append(r)
        # merged direct-out tile (rows NBs*P ...): gate = full g0+g1
        for i, t in enumerate(direct_toks[c]):
            r = NBs * P + i
            grow_flat[r] = gsum[t]
            src_tok[r] = t
        ready = np.empty(len(toks_c), np.int64)
        for j, t in enumerate(toks_c):
            ready[j] = max(r // P for r in row_of[t])
        order = np.argsort(ready, kind="stable")
        toks_sorted = toks_c[order]
        npad = NGATH - len(toks_sorted)
        assert npad >= 0, (c, len(toks_sorted))
        # pad slots first (ready immediately), real tokens after
        gtoks = np.concatenate([np.full(npad, -1, np.int64), toks_sorted])
        gready = np.concatenate([np.zeros(npad, np.int64), ready[order]])
        r0_flat = np.zeros(NGATH, np.int16)
        r1_flat = np.zeros(NGATH, np.int16)
        for pos, t in enumerate(gtoks):
            if t < 0:
                continue
            rs = row_of[t]
            r0_flat[pos] = rs[0]
            r1_flat[pos] = rs[-1] if len(rs) > 1 else rs[0]
        bounds = np.cumsum(CHUNKS) - 1
        ready_all[c] = gready[bounds]
        dpad = np.full(NDIR, -1, np.int64)
        dpad[:len(direct_toks[c])] = direct_toks[c]
        token_ids.append((gtoks, dpad))

        per_core.append(dict(
            grow=np.ascontiguousarray(grow_flat.reshape(NB, P).T),
            r0i=_pack16(r0_flat),
            r1i=_pack16(r1_flat),
            src_tok=src_tok,                       # consumed by _prep
            slot_experts=np.array(slots[c] + [MEXP[c]]),
        ))

    # shared per-chunk readiness tiles
    RTs = ready_all.max(axis=0)
    RTs = np.maximum.accumulate(RTs)
    return T, RTs, per_core, token_ids


def _build_nc(T, RTs):
    NBs = int(T.sum())
    NB = NBs + 1                         # + merged direct-out tile
    off_tiles = np.concatenate([[0], np.cumsum(T)])

    nc = bacc.Bacc("TRN2", target_bir_lowering=False, debug=False,
                   num_devices=NCORES)

    xg = nc.dram_tensor("xg", [P, NB, KT, P], F16, kind="ExternalInput")
    wh = nc.dram_tensor("wh", [NSLOT + 1, P, KT, D_OUT], F16,
                        kind="ExternalInput")
    grow = nc.dram_tensor("grow", [P, NB], F32, kind="ExternalInput")
    r0i = nc.dram_tensor("r0i", [P, NGATH // 16], I16, kind="ExternalInput")
    r1i = nc.dram_tensor("r1i", [P, NGATH // 16], I16, kind="ExternalInput")
    xwarm = nc.dram_tensor("xwarm", [P, P], F16, kind="ExternalInput")
    outT = nc.dram_tensor("outT", [P, (D_OUT // P) * NOUT], F16,
                          kind="ExternalOutput")

    # Pre-TileContext warmup: the first DMAGatherAnt triggers a ~15us Q7
    # extended-instruction library fetch; start it at t=0 so it overlaps
    # the input DMAs and the first matmul tiles.
    warm_idx = nc.alloc_sbuf_tensor("warm_idx", [P, 8], I16)
    warm_dst = nc.alloc_sbuf_tensor("warm_dst", [P, P], F16)
    warm_sem = nc.alloc_semaphore("warm_set")
    warm_dma = nc.alloc_semaphore("warm_dma")
    nc.gpsimd.memset(warm_idx.ap(), 0).then_inc(warm_sem, 1)
    nc.gpsimd.wait_ge(warm_sem, 1)
    nc.gpsimd.dma_gather(
        warm_dst.ap().rearrange("p (a b) -> p a b", a=1),
        xwarm[:].rearrange("n (a b) -> (n a) b", b=P),
        warm_idx.ap(), num_idxs=P, num_idxs_reg=P, elem_size=P,
        transpose=True).then_inc(warm_dma, 16)

    # chunk schedule: (chunk index, token base, size, ready tile)
    chunk_q = []
    base = 0
    for ci, ch in enumerate(CHUNKS):
        chunk_q.append((ci, base, ch, int(RTs[ci])))
        base += ch

    with tile.TileContext(nc) as tc:
        with (
            tc.tile_pool(name="const", bufs=1) as kpool,
            tc.tile_pool(name="w", bufs=3) as wpool,
            tc.tile_pool(name="xT", bufs=1) as xpool,
            tc.tile_pool(name="y", bufs=1) as ypool,
            tc.tile_pool(name="cmb", bufs=4) as cpool,
            tc.tile_pool(name="ot", bufs=2) as opool,
            tc.tile_pool(name="ps", bufs=6, space="PSUM") as ppool,
        ):
            x_t = xpool.tile([P, NB, KT, P], F16)
            w_ts = []
            # critical path first: slot-0 k0 weights + first x tile
            w0 = wpool.tile([P, KT, D_OUT], F16, tag="w")
            w_ts.append(w0)
            nc.scalar.dma_start(w0[:, 0], wh[0, :, 0])
            for kk in range(KT):
                nc.sync.dma_start(x_t[:, 0, kk], xg[:, 0, kk])
            for kk in range(1, KT):
                nc.scalar.dma_start(w0[:, kk], wh[0, :, kk])

            r0_t = kpool.tile([P, NGATH // 16], I16)
            nc.sync.dma_start(r0_t[:], r0i[:])
            r1_t = kpool.tile([P, NGATH // 16], I16)
            nc.sync.dma_start(r1_t[:], r1i[:])
            grow_t = kpool.tile([P, NB], F32)
            nc.sync.dma_start(grow_t[:], grow[:])

            y_t = ypool.tile([P, NB, D_OUT], F16)

            gathered = {}

            def emit_gathers(ci, cbase, csize, rt):
                tag = "b" if csize == 256 else "s"
                g0 = cpool.tile([P, D_OUT // P, csize], F16, tag=f"g0{tag}")
                g1 = cpool.tile([P, D_OUT // P, csize], F16, tag=f"g1{tag}")
                src = y_t[:, :rt + 1, :]
                for dst, ridx in ((g0, r0_t), (g1, r1_t)):
                    nc.gpsimd.dma_gather(
                        dst[:], src,
                        ridx[:, cbase // 16:(cbase + csize) // 16],
                        num_idxs=csize, num_idxs_reg=csize,
                        elem_size=D_OUT, transpose=True,
                        sbuf_tokens_per_rank=P,
                        sbuf_free_dim_per_rank=D_OUT * 2,
                    )
                gathered[ci] = (g0, g1, cbase, csize)

            qi = 0
            for s in range(NSLOT + 1):
                if s == 0:
                    w_t = w_ts[0]
                else:
                    w_t = wpool.tile([P, KT, D_OUT], F16, tag="w")
                    for kk in range(KT):
                        nc.scalar.dma_start(w_t[:, kk], wh[s, :, kk])
                # stream this slot's x tiles (consumption order)
                if s < NSLOT:
                    lo = int(off_tiles[s]) if s > 0 else 1
                    hi = int(off_tiles[s + 1])
                else:
                    lo, hi = NBs, NB
                for g in range(lo, hi):
                    nc.sync.dma_start(x_t[:, g], xg[:, g])
                for g in range(int(off_tiles[s]) if s < NSLOT else NBs, hi):
                    ps0 = ppool.tile([P, 512], F32, tag="ps")
                    ps1 = ppool.tile([P, 512], F32, tag="ps")
                    for kk in range(KT):
                        lhsT = x_t[:, g, kk, :]
                        nc.tensor.matmul(ps0[:], lhsT, w_t[:, kk, 0:512],
                                         start=(kk == 0), stop=(kk == KT - 1))
                        nc.tensor.matmul(ps1[:], lhsT, w_t[:, kk, 512:1024],
                                         start=(kk == 0), stop=(kk == KT - 1))
                    gsc = grow_t[:, g:g + 1]
                    nc.vector.tensor_scalar_mul(y_t[:, g, 0:512], ps0[:], gsc)
                    nc.vector.tensor_scalar_mul(y_t[:, g, 512:1024],
                                                ps1[:], gsc)
                    while qi < len(chunk_q) and chunk_q[qi][3] <= g:
                        emit_gathers(*chunk_q[qi])
                        qi += 1
            while qi < len(chunk_q):
                emit_gathers(*chunk_q[qi])
                qi += 1
            # merged direct-out tile: evicted y rows ARE output rows
            # (scalar engine stream: no head-of-line with chunk out DMAs)
            nc.scalar.dma_start(
                outT[:, NGATH * (D_OUT // P):NOUT * (D_OUT // P)],
                y_t[:, NB - 1, :])
            # adds on vector AFTER all evictions (no head-of-line blocking
            # of evictions); out DMA per chunk
            for ci in sorted(gathered):
                g0, g1, cbase, csize = gathered[ci]
                tag = "b" if csize == 256 else "s"
                ot = opool.tile([P, D_OUT // P, csize], F16, tag=f"ot{tag}")
                nc.vector.tensor_add(out=ot[:], in0=g0[:], in1=g1[:])
                nc.sync.dma_start(
                    outT[:, cbase * (D_OUT // P):
                         (cbase + csize) * (D_OUT // P)],
                    ot[:].rearrange("p a b -> p (a b)"))

    nc.compile()
    return nc


def _prep(inputs):
    x = np.asarray(inputs["input"], np.float32)
    w = np.asarray(inputs["weight"], np.float32)
    k = int(np.asarray(inputs["k"]))
    assert k == TOPK
    sei = np.asarray(inputs["sorted_expert_indices"]).astype(np.int64)
    ssi = np.asarray(inputs["sorted_scattered_indices"]).astype(np.int64)
    gates = np.asarray(inputs["gates"], np.float32)

    tok = ssi // k
    g_row = gates.reshape(-1)[ssi]
    order_by_tok = np.argsort(tok, kind="stable")
    te = sei[order_by_tok].reshape(N_TOK, TOPK)
    tg = g_row[order_by_tok].reshape(N_TOK, TOPK)

    T, RTs, per_core, token_ids = _plan(te, tg)
    NB = int(T.sum()) + 1
    NP = NB * P

    xh = x.astype(np.float16)
    whp = np.ascontiguousarray(
        w.reshape(N_EXP, KT, P, D_OUT).transpose(0, 2, 1, 3)
    ).astype(np.float16)               # [E, P, KT, D_OUT]
    xwarm = np.zeros((P, P), np.float16)

    in_maps = []
    for c in range(NCORES):
        pc = per_core[c]
        src = pc.pop("src_tok")
        se = pc.pop("slot_experts")
        A = np.zeros((NP, D_IN), np.float16)
        m = src >= 0
        A[m] = xh[src[m]]
        AT = np.ascontiguousarray(A.T)                  # [D_IN, NP]
        xg = np.ascontiguousarray(
            AT.reshape(KT, P, NB, P).transpose(1, 2, 0, 3))
        in_maps.append(dict(
            xg=xg,
            wh=np.ascontiguousarray(whp[se]),
            grow=pc["grow"], r0i=pc["r0i"], r1i=pc["r1i"],
            xwarm=xwarm,
        ))
    return T, RTs, in_maps, token_ids


def _run(inputs, trace=False, trace_kwargs=None):
    T, RTs, in_maps, token_ids = _prep(inputs)
    nc = _build_nc(T, RTs)
    res = run_bass_kernel_spmd(
        nc, in_maps, core_ids=list(range(NCORES)), trace=trace,
        **(trace_kwargs or {}),
    )
    out = np.zeros((N_TOK, D_OUT), np.float32)
    for c in range(NCORES):
        oT = res.results[c]["outT"]                      # [P, 8*NOUT]
        gtoks, dtoks = token_ids[c]
        base = 0
        for ch in CHUNKS:
            blk = oT[:, base * (D_OUT // P):(base + ch) * (D_OUT // P)]
            rows = blk.reshape(P, D_OUT // P, ch).transpose(2, 1, 0)
            ids = gtoks[base:base + ch]
            m = ids >= 0
            out[ids[m]] = rows.reshape(ch, D_OUT)[m]
            base += ch
        dblk = oT[:, NGATH * (D_OUT // P):]              # [P, D_OUT]
        m = dtoks >= 0
        out[dtoks[m]] = dblk[m]
    return out, res


def kernel(**inputs) -> np.ndarray:
    out, _ = _run(inputs, trace=bool(int(os.environ.get("KERNEL_TRACE", "0"))))
    return out
